# revision 39
# baseline (speedup 1.0000x reference)
"""Trainium2 Bass kernel for nn_MinimumSpanningTree.

Contract: kernel(**inputs) takes the FULL inputs (guide_in [8, 64, 256, 256]
f32) and returns the FULL output (tree [8, 65535, 2] int32).

Strategy (data-parallel over batch, one image per NeuronCore):
  Device computes the edge-weight distances via the algebraic identity
      d[p, p+k] = sum_c (x[c,p] - x[c,p+k])^2
                = s[p] + s[p+k] - 2 * g_k[p]
  with s[p] = sum_c x^2 and g_k[p] = sum_c x[c,p] x[c,p+k]:
    - ACT: squares (f32 -> fp16) + a share of the f32->fp16 converts
    - DVE: fp16 elementwise products (2x perf mode) + combines
    - Pool: a share of converts/products
    - PE:  all channel reductions as ones-moving stationary matmuls into
           pixel-major PSUM banks (f32 accumulate)
  Host: Boruvka MST per image (exact port of the reference) + assembly.
  The one partition-misaligned seam (image row 127 of d_row) is patched on
  the host exactly from guide_in.

Self-contained: shapes/sharding hardcoded.
"""
import numpy as np

B, C, H, W = 8, 64, 256, 256
V = H * W
E_ROW = (H - 1) * W
E_COL = H * (W - 1)
E = E_ROW + E_COL
N_ROUNDS = 16

PAD = 260
CHUNK = 2048
NPAIR = 16            # pair pc holds chunk pc (parts 0:64) + chunk pc+16 (64:128)
NBLK = CHUNK // 128   # 16 stationary blocks per chunk

_compiled = None


def _build_program():
    import concourse.bacc as bacc
    import concourse.mybir as mybir
    from concourse import tile

    F32 = mybir.dt.float32
    F16 = mybir.dt.float16
    AL = mybir.AluOpType
    ACT = mybir.ActivationFunctionType

    nc = bacc.Bacc('TRN2', target_bir_lowering=False, debug=False, num_devices=8)
    d_fm = nc.dram_tensor("fm", [C, V + PAD], F32, kind="ExternalInput")
    # cols 0:512 = d_row sums bank, 512:1024 = d_col sums bank
    # value at [m, j] is the distance for pixel p = 128*j + m
    o_d = nc.dram_tensor("dout", [128, 1024], F32, kind="ExternalOutput")

    # engine assignment per pair (conv: ACT x7 / Pool x8 / DVE for the
    # last-processed pair 13; products all on DVE). Pairs 14, 15 are loaded
    # and processed FIRST so the post-final-DMA tail is pair 13 only.
    conv_eng = (['p', 'a', 'p', 'a', 'p', 'p', 'a', 'p', 'p', 'p', 'p', 'p',
                 'v', 'v', 'a', 'a'])
    pcol_eng = ['v'] * 16
    tail_eng = ['v'] * 16
    ORDER = [14, 15] + list(range(14))

    with tile.TileContext(nc) as tc:
        with tc.tile_pool(name="xp", bufs=6) as xp, \
             tc.tile_pool(name="wp", bufs=4) as wp, \
             tc.tile_pool(name="cst", bufs=1) as cst, \
             tc.tile_pool(name="ps", bufs=1, space="PSUM") as psum:

            # moving operands for the stationary matmuls
            m_s = cst.tile([128, 2], F16)    # +1 on chunk-half diagonal blocks
            m_g = cst.tile([128, 2], F16)    # -2 pattern
            nc.vector.memset(m_s[:], 0.0)
            nc.vector.memset(m_g[:], 0.0)
            nc.vector.memset(m_s[0:64, 0:1], 1.0)
            nc.vector.memset(m_s[64:128, 1:2], 1.0)
            nc.vector.memset(m_g[0:64, 0:1], -2.0)
            nc.vector.memset(m_g[64:128, 1:2], -2.0)

            # accumulation banks (pixel-major): [m, j] = pixel 128j+m, with
            # matmul groups accumulating d = s + s_shift - 2 g in place.
            ps_dr = psum.tile([128, 512], F32)
            ps_dc = psum.tile([128, 512], F32)

            # per-quarter out staging: [128, 4 ranges x 64 cols]; range t of
            # quarter q maps to o_d cols 256t + 64q + [0,64)
            douts = []
            for q in range(4):
                doq = cst.tile([128, 256], F32, tag=f"do{q}")
                douts.append(doq)

            def load_pair(pc):
                x = xp.tile([128, CHUNK + 1], F32, tag="x")
                a0 = pc * CHUNK
                b0 = (pc + 16) * CHUNK
                if pc == 13:
                    # sliced load: the last-processed pair streams in 512-col
                    # slices so its compute pipelines with the final DMAs
                    for s in range(4):
                        w = 513 if s == 3 else 512
                        o = 512 * s
                        nc.sync.dma_start(x[0:64, o:o + w], d_fm[:, a0 + o: a0 + o + w])
                        nc.sync.dma_start(x[64:128, o:o + w], d_fm[:, b0 + o: b0 + o + w])
                else:
                    nc.sync.dma_start(x[0:64, :], d_fm[:, a0: a0 + CHUNK + 1])
                    nc.sync.dma_start(x[64:128, :], d_fm[:, b0: b0 + CHUNK + 1])
                return x

            def conv_pair(pc, x):
                x16 = xp.tile([128, CHUNK + 1], F16, tag="x16")
                e = conv_eng[pc]
                if e == 'v':
                    nc.vector.tensor_copy(x16[:], x[:])
                elif e == 'a':
                    nc.scalar.copy(x16[:], x[:])
                else:
                    nc.gpsimd.tensor_copy(x16[:], x[:])
                return x16

            def pcols(ps, col):
                # strided pair of bank columns {col, col+256} (chunkA, chunkB)
                return ps[:].rearrange("p (t c) -> p t c", t=2)[:, :, col:col + 1]

            def emit_quarter(q, engs='va'):
                # copy finished quarters of the PSUM banks to the staging
                # tile; quarter q covers bank cols [64q,64q+64) + [256+64q,..)
                lo = 64 * q
                for ps, base, eng in ((ps_dr, 0, engs[0]), (ps_dc, 2, engs[1])):
                    src = ps[:].rearrange("p (t c) -> p t c", t=2)[:, :, lo:lo + 64]
                    dst = douts[q][:].rearrange("p (t c) -> p t c", t=4)[:, base:base + 2, :]
                    if eng == 'v':
                        nc.vector.tensor_copy(dst, src)
                    elif eng == 'p':
                        nc.gpsimd.tensor_copy(dst, src)
                    else:
                        nc.scalar.copy(dst, src)

            def out_quarter(q):
                # one DMA per quarter covering both banks (4 ranges of 64 cols)
                lo = 64 * q
                out = o_d[:].rearrange("p (t c) -> p t c", t=4)[:, :, lo:lo + 64]
                src = douts[q][:].rearrange("p (t c) -> p t c", t=4)
                nc.sync.dma_start(out, src)

            def conv_pair_ded(pc, x):
                # pairs 14/15: x16 in a dedicated tile (read again at the end)
                x16 = cst.tile([128, CHUNK + 1], F16, tag=f"x16d{pc}")
                if conv_eng[pc] == 'a':
                    nc.scalar.copy(x16[:], x[:])
                else:
                    nc.gpsimd.tensor_copy(x16[:], x[:])
                return x16

            xs = [None] * NPAIR
            x16s = [None] * NPAIR
            xxs = [None] * NPAIR
            prs = [None] * NPAIR

            xs[14] = load_pair(14)
            x16s[14] = conv_pair_ded(14, xs[14])

            def emit_tail(pc):
                # pr tail products for pair pc (partner = next chunk's head,
                # which lives in x16 of pair pc+1; pair 15 wraps to itself --
                # the affected outputs are host-patched / out of range).
                # Emitted one pair late so the partner conv is long done and
                # DVE never stalls on it.
                part = x16s[pc + 1] if pc + 1 < NPAIR else x16s[pc]
                nc.vector.tensor_tensor(prs[pc][:, 1792:2048],
                                        x16s[pc][:, 1792:2048],
                                        part[:, 0:256], AL.mult)

            def dr_group(col, full):
                q = col // 16
                bs = col % 16
                blk = slice(128 * bs, 128 * bs + 128)
                b2 = (col + 2) % 16
                blk2 = slice(128 * b2, 128 * b2 + 128)
                q2 = (col + 2) // 16
                nc.tensor.matmul(pcols(ps_dr, col), xxs[q][:, blk],
                                 m_s[:], start=True, stop=False)
                nc.tensor.matmul(pcols(ps_dr, col), prs[q][:, blk],
                                 m_g[:], start=False, stop=not full)
                if full:
                    nc.tensor.matmul(pcols(ps_dr, col), xxs[q2][:, blk2],
                                     m_s[:], start=False, stop=True)

            for i, pc in enumerate(ORDER):
                if i + 1 < NPAIR:
                    nxt = ORDER[i + 1]
                    xs[nxt] = load_pair(nxt)
                    if nxt == 15:
                        x16s[nxt] = conv_pair_ded(nxt, xs[nxt])
                    elif nxt == 13:
                        # only slice 0 converted at prefetch (for pr-tail(12));
                        # remaining slices interleave into body(13)
                        x16t = xp.tile([128, CHUNK + 1], F16, tag="x16")
                        x16s[nxt] = x16t
                        nc.vector.tensor_copy(x16t[:, 0:512],
                                              xs[nxt][:, 0:512])
                    else:
                        x16s[nxt] = conv_pair(nxt, xs[nxt])
                x = xs[pc]
                x16 = x16s[pc]
                x16n = x16s[pc + 1] if pc + 1 < NPAIR else x16s[pc]

                if pc in (14, 15):
                    xx = cst.tile([128, CHUNK + 1], F16, tag=f"xxd{pc}")
                else:
                    xx = wp.tile([128, CHUNK + 1], F16, tag="xx")
                xxs[pc] = xx

                if pc == 15:
                    pr = cst.tile([128, CHUNK], F16, tag="prd15")
                else:
                    pr = wp.tile([128, CHUNK], F16, tag="pr")
                prs[pc] = pr
                pcl = wp.tile([128, CHUNK], F16, tag="pc")

                if pc == 13:
                    # sliced, interleaved so DVE work pipelines with the final
                    # slice DMAs: conv(s+1) -> sq(s) -> pr(s) -> pcl(s)
                    for s in range(4):
                        o = 512 * s
                        w = 513 if s == 3 else 512
                        if s < 3:
                            w2 = 513 if s == 2 else 512
                            nc.vector.tensor_copy(
                                x16[:, o + 512:o + 512 + w2],
                                x[:, o + 512:o + 512 + w2])
                        nc.scalar.activation(xx[:, o:o + w], x[:, o:o + w],
                                             ACT.Square)
                        if s < 3:
                            nc.vector.tensor_tensor(
                                pr[:, o:o + 512], x16[:, o:o + 512],
                                x16[:, o + 256:o + 768], AL.mult)
                        else:
                            nc.vector.tensor_tensor(
                                pr[:, o:o + 256], x16[:, o:o + 256],
                                x16[:, o + 256:o + 512], AL.mult)
                            nc.vector.tensor_tensor(
                                pr[:, 1792:2048], x16[:, 1792:2048],
                                x16n[:, 0:256], AL.mult)
                        ep = nc.gpsimd if s < 2 else nc.vector
                        ep.tensor_tensor(
                            pcl[:, o:o + 512], x16[:, o:o + 512],
                            x16[:, o + 1:o + 513], AL.mult)
                else:
                    nc.scalar.activation(xx[:], x[:], ACT.Square)
                    nc.vector.tensor_tensor(pr[:, 0:1792], x16[:, 0:1792],
                                            x16[:, 256:2048], AL.mult)
                    ec = nc.vector if pcol_eng[pc] == 'v' else nc.gpsimd
                    ec.tensor_tensor(pcl[:], x16[:, 0:CHUNK],
                                     x16[:, 1:CHUNK + 1], AL.mult)

                # deferred tail products (see emit_tail)
                if pc == 15:
                    emit_tail(14)
                    emit_tail(15)
                elif 1 <= pc <= 13:
                    emit_tail(pc - 1)

                # d_col groups (contiguous per column: s -> s_shift1 -> -2*g)
                for b in range(NBLK):
                    col = 16 * pc + b
                    blk = slice(128 * b, 128 * b + 128)
                    nc.tensor.matmul(pcols(ps_dc, col), xx[:, blk], m_s[:],
                                     start=True, stop=False)
                    nc.tensor.matmul(pcols(ps_dc, col),
                                     xx[:, 128 * b + 1:128 * b + 129], m_s[:],
                                     start=False, stop=False)
                    nc.tensor.matmul(pcols(ps_dc, col), pcl[:, blk], m_g[:],
                                     start=False, stop=True)

                # d_row groups whose s' source (pixel+256 = col+2) lives in
                # this pair's xx. Deferred to the end: cols 222-223 (s' in
                # xx(14) but s/g in pair 13, processed last) and the seam.
                if pc == 14:
                    cols = range(224, 238)
                elif pc == 15:
                    cols = range(238, 254)
                else:
                    cols = range(max(0, 16 * pc - 2), 16 * pc + 14)
                for col in cols:
                    dr_group(col, True)

                if pc == 4:
                    emit_quarter(0)
                elif pc == 8:
                    emit_quarter(1)
                elif pc == 12:
                    emit_quarter(2, engs='aa')

            # deferred cols: 222/223 full groups; 254/255 have no valid s'
            # (partition-misaligned seam, host patches image row 127 / beyond
            # the image) -- groups end at g.
            for col in (222, 223):
                dr_group(col, True)
            for col in (254, 255):
                dr_group(col, False)

            emit_quarter(3, engs='aa')
            out_quarter(0)
            out_quarter(1)
            out_quarter(2)
            out_quarter(3)

    nc.compile()
    return nc


def _get_program():
    global _compiled
    if _compiled is None:
        _compiled = _build_program()
    return _compiled


def _edge_weights_device(guide_in):
    """Run the bass program on 8 cores; returns (wr [B,255,256], wc [B,256,255])."""
    from concourse.bass_utils import run_bass_kernel_spmd

    nc = _get_program()
    pad = np.zeros((C, PAD), np.float32)
    in_maps = []
    for b in range(B):
        fm = np.ascontiguousarray(guide_in[b].reshape(C, V))
        in_maps.append({"fm": np.concatenate([fm, pad], axis=1)})
    res = run_bass_kernel_spmd(nc, in_maps, list(range(8)))

    wr, wc = [], []
    for b in range(B):
        r = np.asarray(res.results[b]["dout"])
        drow = r[:, 0:512].T.reshape(-1)       # [V], pixel-ordered
        dcol = r[:, 512:1024].T.reshape(-1)
        drow = drow[:E_ROW].reshape(H - 1, W).copy()
        # patch the partition-misaligned seam (image row 127) exactly
        g = guide_in[b]
        drow[127, :] = ((g[:, 127, :] - g[:, 128, :]) ** 2).sum(0)
        dcol = dcol.reshape(H, W)[:, :W - 1]
        wr.append(drow + np.float32(1.0))
        wc.append(dcol + np.float32(1.0))
    return np.stack(wr), np.stack(wc)


def _build_index():
    raw = np.arange(V, dtype=np.int32).reshape(H, W)
    row_e = np.stack([raw[:-1, :], raw[1:, :]], axis=-1).reshape(-1, 2)
    col_e = np.stack([raw[:, :-1], raw[:, 1:]], axis=-1).reshape(-1, 2)
    return np.concatenate([row_e, col_e], axis=0)


def _scatter_min(target, keys, vals):
    order = np.argsort(keys, kind="stable")
    ks = keys[order]
    vs = vals[order]
    starts = np.flatnonzero(np.r_[True, ks[1:] != ks[:-1]])
    mins = np.minimum.reduceat(vs, starts)
    target[ks[starts]] = np.minimum(target[ks[starts]], mins)


def _mst_boruvka(u, v, w):
    """Exact port of the reference Boruvka (per image)."""
    eidx = np.arange(E, dtype=np.int64)
    vidx = np.arange(V, dtype=np.int64)
    INF = np.float32(np.inf)
    BIGE = E
    comp = vidx.copy()
    sel = np.zeros(E, dtype=bool)
    for _ in range(N_ROUNDS):
        cu, cv = comp[u], comp[v]
        active = cu != cv
        if not active.any():
            break
        wa = np.where(active, w, INF)
        minw = np.full(V, INF, np.float32)
        _scatter_min(minw, cu, wa)
        _scatter_min(minw, cv, wa)
        cand_u = np.where(active & (wa == minw[cu]), eidx, BIGE)
        cand_v = np.where(active & (wa == minw[cv]), eidx, BIGE)
        best = np.full(V, BIGE, np.int64)
        _scatter_min(best, cu, cand_u)
        _scatter_min(best, cv, cand_v)
        has = best < BIGE
        be = np.clip(best, 0, E - 1)
        cu_b, cv_b = comp[u[be]], comp[v[be]]
        parent = np.where(has, np.where(cu_b == vidx, cv_b, cu_b), vidx)
        pp = parent[parent]
        parent = np.where((pp == vidx) & (vidx < parent), vidx, parent)
        for _ in range(N_ROUNDS):
            parent = parent[parent]
        comp = parent[comp]
        sel_idx = best[has]
        sel[sel_idx] = True
    return sel


def kernel(guide_in):
    guide_in = np.asarray(guide_in, dtype=np.float32)
    wr, wc = _edge_weights_device(guide_in)

    index = _build_index()
    u = index[:, 0].astype(np.int64)
    v = index[:, 1].astype(np.int64)
    trees = []
    for b in range(B):
        w = np.concatenate([wr[b].reshape(-1), wc[b].reshape(-1)]).astype(np.float32)
        sel = _mst_boruvka(u, v, w)
        eids = np.nonzero(sel)[0]
        if len(eids) != V - 1:
            eids = np.concatenate([eids, np.zeros(max(0, V - 1 - len(eids)), np.int64)])[:V - 1]
        trees.append(index[eids])
    return np.stack(trees).astype(np.int32)


# revision 40
# speedup vs baseline: 1.0003x; 1.0003x over previous
"""Trainium2 Bass kernel for nn_MinimumSpanningTree.

Contract: kernel(**inputs) takes the FULL inputs (guide_in [8, 64, 256, 256]
f32) and returns the FULL output (tree [8, 65535, 2] int32).

Strategy (data-parallel over batch, one image per NeuronCore):
  Device computes the edge-weight distances via the algebraic identity
      d[p, p+k] = sum_c (x[c,p] - x[c,p+k])^2
                = s[p] + s[p+k] - 2 * g_k[p]
  with s[p] = sum_c x^2 and g_k[p] = sum_c x[c,p] x[c,p+k]:
    - ACT: squares (f32 -> fp16) + a share of the f32->fp16 converts
    - DVE: fp16 elementwise products (2x perf mode) + combines
    - Pool: a share of converts/products
    - PE:  all channel reductions as ones-moving stationary matmuls into
           pixel-major PSUM banks (f32 accumulate)
  Host: Boruvka MST per image (exact port of the reference) + assembly.
  The one partition-misaligned seam (image row 127 of d_row) is patched on
  the host exactly from guide_in.

Self-contained: shapes/sharding hardcoded.
"""
import numpy as np

B, C, H, W = 8, 64, 256, 256
V = H * W
E_ROW = (H - 1) * W
E_COL = H * (W - 1)
E = E_ROW + E_COL
N_ROUNDS = 16

PAD = 260
CHUNK = 2048
NPAIR = 16            # pair pc holds chunk pc (parts 0:64) + chunk pc+16 (64:128)
NBLK = CHUNK // 128   # 16 stationary blocks per chunk

_compiled = None


def _build_program():
    import concourse.bacc as bacc
    import concourse.mybir as mybir
    from concourse import tile

    F32 = mybir.dt.float32
    F16 = mybir.dt.float16
    AL = mybir.AluOpType
    ACT = mybir.ActivationFunctionType

    nc = bacc.Bacc('TRN2', target_bir_lowering=False, debug=False, num_devices=8)
    d_fm = nc.dram_tensor("fm", [C, V + PAD], F32, kind="ExternalInput")
    # cols 0:512 = d_row sums bank, 512:1024 = d_col sums bank
    # value at [m, j] is the distance for pixel p = 128*j + m
    o_d = nc.dram_tensor("dout", [128, 1024], F32, kind="ExternalOutput")

    # engine assignment per pair (conv: ACT x7 / Pool x8 / DVE for the
    # last-processed pair 13; products all on DVE). Pairs 14, 15 are loaded
    # and processed FIRST so the post-final-DMA tail is pair 13 only.
    conv_eng = (['p', 'a', 'p', 'a', 'p', 'p', 'a', 'p', 'p', 'p', 'p', 'p',
                 'v', 'v', 'a', 'a'])
    pcol_eng = ['v'] * 16
    tail_eng = ['v'] * 16
    ORDER = [14, 15] + list(range(14))

    with tile.TileContext(nc) as tc:
        with tc.tile_pool(name="xp", bufs=6) as xp, \
             tc.tile_pool(name="wp", bufs=4) as wp, \
             tc.tile_pool(name="cst", bufs=1) as cst, \
             tc.tile_pool(name="ps", bufs=1, space="PSUM") as psum:

            # moving operands for the stationary matmuls
            m_s = cst.tile([128, 2], F16)    # +1 on chunk-half diagonal blocks
            m_g = cst.tile([128, 2], F16)    # -2 pattern
            nc.vector.memset(m_s[:], 0.0)
            nc.vector.memset(m_g[:], 0.0)
            nc.vector.memset(m_s[0:64, 0:1], 1.0)
            nc.vector.memset(m_s[64:128, 1:2], 1.0)
            nc.vector.memset(m_g[0:64, 0:1], -2.0)
            nc.vector.memset(m_g[64:128, 1:2], -2.0)

            # accumulation banks (pixel-major): [m, j] = pixel 128j+m, with
            # matmul groups accumulating d = s + s_shift - 2 g in place.
            ps_dr = psum.tile([128, 512], F32)
            ps_dc = psum.tile([128, 512], F32)

            # per-quarter out staging: [128, 4 ranges x 64 cols]; range t of
            # quarter q maps to o_d cols 256t + 64q + [0,64)
            douts = []
            for q in range(4):
                doq = cst.tile([128, 256], F32, tag=f"do{q}")
                douts.append(doq)

            def load_pair(pc):
                x = xp.tile([128, CHUNK + 1], F32, tag="x")
                a0 = pc * CHUNK
                b0 = (pc + 16) * CHUNK
                if pc == 13:
                    # sliced load: the last-processed pair streams in 512-col
                    # slices so its compute pipelines with the final DMAs
                    for s in range(4):
                        w = 513 if s == 3 else 512
                        o = 512 * s
                        nc.sync.dma_start(x[0:64, o:o + w], d_fm[:, a0 + o: a0 + o + w])
                        nc.sync.dma_start(x[64:128, o:o + w], d_fm[:, b0 + o: b0 + o + w])
                else:
                    nc.sync.dma_start(x[0:64, :], d_fm[:, a0: a0 + CHUNK + 1])
                    nc.sync.dma_start(x[64:128, :], d_fm[:, b0: b0 + CHUNK + 1])
                return x

            def conv_pair(pc, x):
                x16 = xp.tile([128, CHUNK + 1], F16, tag="x16")
                e = conv_eng[pc]
                if e == 'v':
                    nc.vector.tensor_copy(x16[:], x[:])
                elif e == 'a':
                    nc.scalar.copy(x16[:], x[:])
                else:
                    nc.gpsimd.tensor_copy(x16[:], x[:])
                return x16

            def pcols(ps, col):
                # strided pair of bank columns {col, col+256} (chunkA, chunkB)
                return ps[:].rearrange("p (t c) -> p t c", t=2)[:, :, col:col + 1]

            def emit_quarter(q, engs='va'):
                # copy finished quarters of the PSUM banks to the staging
                # tile; quarter q covers bank cols [64q,64q+64) + [256+64q,..)
                lo = 64 * q
                for ps, base, eng in ((ps_dr, 0, engs[0]), (ps_dc, 2, engs[1])):
                    src = ps[:].rearrange("p (t c) -> p t c", t=2)[:, :, lo:lo + 64]
                    dst = douts[q][:].rearrange("p (t c) -> p t c", t=4)[:, base:base + 2, :]
                    if eng == 'v':
                        nc.vector.tensor_copy(dst, src)
                    elif eng == 'p':
                        nc.gpsimd.tensor_copy(dst, src)
                    else:
                        nc.scalar.copy(dst, src)

            def out_quarter(q):
                # one DMA per quarter covering both banks (4 ranges of 64 cols)
                lo = 64 * q
                out = o_d[:].rearrange("p (t c) -> p t c", t=4)[:, :, lo:lo + 64]
                src = douts[q][:].rearrange("p (t c) -> p t c", t=4)
                nc.sync.dma_start(out, src)

            def conv_pair_ded(pc, x):
                # pairs 14/15: x16 in a dedicated tile (read again at the end)
                x16 = cst.tile([128, CHUNK + 1], F16, tag=f"x16d{pc}")
                if conv_eng[pc] == 'a':
                    nc.scalar.copy(x16[:], x[:])
                else:
                    nc.gpsimd.tensor_copy(x16[:], x[:])
                return x16

            xs = [None] * NPAIR
            x16s = [None] * NPAIR
            xxs = [None] * NPAIR
            prs = [None] * NPAIR

            xs[14] = load_pair(14)
            x16s[14] = conv_pair_ded(14, xs[14])

            def emit_tail(pc):
                # pr tail products for pair pc (partner = next chunk's head,
                # which lives in x16 of pair pc+1; pair 15 wraps to itself --
                # the affected outputs are host-patched / out of range).
                # Emitted one pair late so the partner conv is long done and
                # DVE never stalls on it.
                part = x16s[pc + 1] if pc + 1 < NPAIR else x16s[pc]
                nc.vector.tensor_tensor(prs[pc][:, 1792:2048],
                                        x16s[pc][:, 1792:2048],
                                        part[:, 0:256], AL.mult)

            def dr_group(col, full):
                q = col // 16
                bs = col % 16
                blk = slice(128 * bs, 128 * bs + 128)
                b2 = (col + 2) % 16
                blk2 = slice(128 * b2, 128 * b2 + 128)
                q2 = (col + 2) // 16
                nc.tensor.matmul(pcols(ps_dr, col), xxs[q][:, blk],
                                 m_s[:], start=True, stop=False)
                nc.tensor.matmul(pcols(ps_dr, col), prs[q][:, blk],
                                 m_g[:], start=False, stop=not full)
                if full:
                    nc.tensor.matmul(pcols(ps_dr, col), xxs[q2][:, blk2],
                                     m_s[:], start=False, stop=True)

            for i, pc in enumerate(ORDER):
                if i + 1 < NPAIR:
                    nxt = ORDER[i + 1]
                    xs[nxt] = load_pair(nxt)
                    if nxt == 15:
                        x16s[nxt] = conv_pair_ded(nxt, xs[nxt])
                    elif nxt == 13:
                        # only slice 0 converted at prefetch (for pr-tail(12));
                        # remaining slices interleave into body(13)
                        x16t = xp.tile([128, CHUNK + 1], F16, tag="x16")
                        x16s[nxt] = x16t
                        nc.vector.tensor_copy(x16t[:, 0:512],
                                              xs[nxt][:, 0:512])
                    else:
                        x16s[nxt] = conv_pair(nxt, xs[nxt])
                x = xs[pc]
                x16 = x16s[pc]
                x16n = x16s[pc + 1] if pc + 1 < NPAIR else x16s[pc]

                if pc in (14, 15):
                    xx = cst.tile([128, CHUNK + 1], F16, tag=f"xxd{pc}")
                else:
                    xx = wp.tile([128, CHUNK + 1], F16, tag="xx")
                xxs[pc] = xx

                if pc == 15:
                    pr = cst.tile([128, CHUNK], F16, tag="prd15")
                else:
                    pr = wp.tile([128, CHUNK], F16, tag="pr")
                prs[pc] = pr
                pcl = wp.tile([128, CHUNK], F16, tag="pc")

                if pc == 13:
                    # sliced, interleaved so DVE work pipelines with the final
                    # slice DMAs: conv(s+1) -> sq(s) -> pr(s) -> pcl(s)
                    for s in range(4):
                        o = 512 * s
                        w = 513 if s == 3 else 512
                        if s < 3:
                            w2 = 513 if s == 2 else 512
                            nc.vector.tensor_copy(
                                x16[:, o + 512:o + 512 + w2],
                                x[:, o + 512:o + 512 + w2])
                        nc.scalar.activation(xx[:, o:o + w], x[:, o:o + w],
                                             ACT.Square)
                        if s < 3:
                            nc.vector.tensor_tensor(
                                pr[:, o:o + 512], x16[:, o:o + 512],
                                x16[:, o + 256:o + 768], AL.mult)
                        else:
                            nc.vector.tensor_tensor(
                                pr[:, o:o + 256], x16[:, o:o + 256],
                                x16[:, o + 256:o + 512], AL.mult)
                            nc.vector.tensor_tensor(
                                pr[:, 1792:2048], x16[:, 1792:2048],
                                x16n[:, 0:256], AL.mult)
                        ep = nc.gpsimd if s < 2 else nc.vector
                        ep.tensor_tensor(
                            pcl[:, o:o + 512], x16[:, o:o + 512],
                            x16[:, o + 1:o + 513], AL.mult)
                else:
                    nc.scalar.activation(xx[:], x[:], ACT.Square)
                    nc.vector.tensor_tensor(pr[:, 0:1792], x16[:, 0:1792],
                                            x16[:, 256:2048], AL.mult)
                    ec = nc.vector if pcol_eng[pc] == 'v' else nc.gpsimd
                    ec.tensor_tensor(pcl[:], x16[:, 0:CHUNK],
                                     x16[:, 1:CHUNK + 1], AL.mult)

                # deferred tail products (see emit_tail)
                if pc == 15:
                    emit_tail(14)
                    emit_tail(15)
                elif 1 <= pc <= 13:
                    emit_tail(pc - 1)

                # d_col groups (contiguous per column: s -> s_shift1 -> -2*g)
                for b in range(NBLK):
                    col = 16 * pc + b
                    blk = slice(128 * b, 128 * b + 128)
                    nc.tensor.matmul(pcols(ps_dc, col), xx[:, blk], m_s[:],
                                     start=True, stop=False)
                    nc.tensor.matmul(pcols(ps_dc, col),
                                     xx[:, 128 * b + 1:128 * b + 129], m_s[:],
                                     start=False, stop=False)
                    nc.tensor.matmul(pcols(ps_dc, col), pcl[:, blk], m_g[:],
                                     start=False, stop=True)

                # d_row groups whose s' source (pixel+256 = col+2) lives in
                # this pair's xx. Deferred to the end: cols 222-223 (s' in
                # xx(14) but s/g in pair 13, processed last) and the seam.
                if pc == 14:
                    cols = range(224, 238)
                elif pc == 15:
                    cols = range(238, 254)
                    # seam cols: no valid s' (host patches image row 127 /
                    # beyond the image) -- sources are this pair's, emit now
                    for col in (254, 255):
                        dr_group(col, False)
                else:
                    cols = range(max(0, 16 * pc - 2), 16 * pc + 14)
                for col in cols:
                    dr_group(col, True)

                if pc == 4:
                    emit_quarter(0)
                elif pc == 8:
                    emit_quarter(1)
                elif pc == 12:
                    emit_quarter(2, engs='aa')

            # deferred cols: 222/223 full groups; 254/255 have no valid s'
            # (partition-misaligned seam, host patches image row 127 / beyond
            # the image) -- groups end at g.
            for col in (222, 223):
                dr_group(col, True)

            emit_quarter(3, engs='aa')
            out_quarter(0)
            out_quarter(1)
            out_quarter(2)
            out_quarter(3)

    nc.compile()
    return nc


def _get_program():
    global _compiled
    if _compiled is None:
        _compiled = _build_program()
    return _compiled


def _edge_weights_device(guide_in):
    """Run the bass program on 8 cores; returns (wr [B,255,256], wc [B,256,255])."""
    from concourse.bass_utils import run_bass_kernel_spmd

    nc = _get_program()
    pad = np.zeros((C, PAD), np.float32)
    in_maps = []
    for b in range(B):
        fm = np.ascontiguousarray(guide_in[b].reshape(C, V))
        in_maps.append({"fm": np.concatenate([fm, pad], axis=1)})
    res = run_bass_kernel_spmd(nc, in_maps, list(range(8)))

    wr, wc = [], []
    for b in range(B):
        r = np.asarray(res.results[b]["dout"])
        drow = r[:, 0:512].T.reshape(-1)       # [V], pixel-ordered
        dcol = r[:, 512:1024].T.reshape(-1)
        drow = drow[:E_ROW].reshape(H - 1, W).copy()
        # patch the partition-misaligned seam (image row 127) exactly
        g = guide_in[b]
        drow[127, :] = ((g[:, 127, :] - g[:, 128, :]) ** 2).sum(0)
        dcol = dcol.reshape(H, W)[:, :W - 1]
        wr.append(drow + np.float32(1.0))
        wc.append(dcol + np.float32(1.0))
    return np.stack(wr), np.stack(wc)


def _build_index():
    raw = np.arange(V, dtype=np.int32).reshape(H, W)
    row_e = np.stack([raw[:-1, :], raw[1:, :]], axis=-1).reshape(-1, 2)
    col_e = np.stack([raw[:, :-1], raw[:, 1:]], axis=-1).reshape(-1, 2)
    return np.concatenate([row_e, col_e], axis=0)


def _scatter_min(target, keys, vals):
    order = np.argsort(keys, kind="stable")
    ks = keys[order]
    vs = vals[order]
    starts = np.flatnonzero(np.r_[True, ks[1:] != ks[:-1]])
    mins = np.minimum.reduceat(vs, starts)
    target[ks[starts]] = np.minimum(target[ks[starts]], mins)


def _mst_boruvka(u, v, w):
    """Exact port of the reference Boruvka (per image)."""
    eidx = np.arange(E, dtype=np.int64)
    vidx = np.arange(V, dtype=np.int64)
    INF = np.float32(np.inf)
    BIGE = E
    comp = vidx.copy()
    sel = np.zeros(E, dtype=bool)
    for _ in range(N_ROUNDS):
        cu, cv = comp[u], comp[v]
        active = cu != cv
        if not active.any():
            break
        wa = np.where(active, w, INF)
        minw = np.full(V, INF, np.float32)
        _scatter_min(minw, cu, wa)
        _scatter_min(minw, cv, wa)
        cand_u = np.where(active & (wa == minw[cu]), eidx, BIGE)
        cand_v = np.where(active & (wa == minw[cv]), eidx, BIGE)
        best = np.full(V, BIGE, np.int64)
        _scatter_min(best, cu, cand_u)
        _scatter_min(best, cv, cand_v)
        has = best < BIGE
        be = np.clip(best, 0, E - 1)
        cu_b, cv_b = comp[u[be]], comp[v[be]]
        parent = np.where(has, np.where(cu_b == vidx, cv_b, cu_b), vidx)
        pp = parent[parent]
        parent = np.where((pp == vidx) & (vidx < parent), vidx, parent)
        for _ in range(N_ROUNDS):
            parent = parent[parent]
        comp = parent[comp]
        sel_idx = best[has]
        sel[sel_idx] = True
    return sel


def kernel(guide_in):
    guide_in = np.asarray(guide_in, dtype=np.float32)
    wr, wc = _edge_weights_device(guide_in)

    index = _build_index()
    u = index[:, 0].astype(np.int64)
    v = index[:, 1].astype(np.int64)
    trees = []
    for b in range(B):
        w = np.concatenate([wr[b].reshape(-1), wc[b].reshape(-1)]).astype(np.float32)
        sel = _mst_boruvka(u, v, w)
        eids = np.nonzero(sel)[0]
        if len(eids) != V - 1:
            eids = np.concatenate([eids, np.zeros(max(0, V - 1 - len(eids)), np.int64)])[:V - 1]
        trees.append(index[eids])
    return np.stack(trees).astype(np.int32)
